# revision 3
# baseline (speedup 1.0000x reference)
"""MDTA (channel-attention transformer block) Trainium2 kernel, V3.

Math (validated against the jax reference):
  z      = (x - mu) * rsqrt(var + eps)     per-token LayerNorm core
  G      = z^T z                           (C x C Gram, contracted over t)
  scores = Wk'^T G Wq' / alpha             (Wq' = diag(gamma) Wq, etc.)
  attn   = blockwise softmax(scores)       (4 diagonal 32x32 blocks)
  W2     = diag(g) Wv blockdiag(attn) Wf + diag(gamma)
  y      = z @ W2

Sharding: 8 cores = (batch b in 0..3) x (token half in 0..1); cores 2b, 2b+1
all-reduce their Gram, then each streams yT = W2^T zT for its token half.

V3 engine assignment (per 8-tile supergroup of 1024 tokens):
  DVE : 8x bn_stats (per-tile mean/var moments) + 8x tensor_scalar z-norm
        + reciprocal. These are the irreducible per-tile ops (~225 ns each).
  Pool: LayerNorm field math on [128,8] strided views of the bn_stats output
        (mu/var assembly) - small ops only, Q7 launch overhead ~200-300 ns.
  ACT : sqrt(var+eps), zT PSUM->SBUF copies, y PSUM->SBUF fp16 copies.
  PE  : Gram accumulation + fp16 transposes (pipelined back-to-back),
        phase-3 512-wide matmuls.
  I/O : x arrives fp16 (host-cast), yT leaves fp16 (host-upcast).

bn_stats output layout (HW): per tile, 6 f32 values
  [n_even, mean_even, n*var_even, n_odd, mean_odd, n*var_odd]
with n_even = n_odd = 64 for a 128-wide tile:
  mu   = (me + mo) / 2
  128*var = (cve + cvo) + 32*(me - mo)^2
"""

import sys

import numpy as np

for _p in ("/opt/trn_rl_repo",):
    if _p not in sys.path:
        sys.path.append(_p)

import concourse.bacc as bacc
import concourse.bass as bass
import concourse.tile as tile
from concourse import mybir
from concourse.bass_utils import run_bass_kernel_spmd

B, HH, WW, C = 4, 256, 256, 128
NH, S = 4, 32
T = HH * WW            # tokens per batch
N_CORES = 8
TLOC = T // 2          # tokens per core
EPS = 1e-5
P = 128                # partitions / tile token count
GRP = 8                # tiles per supergroup
YCHUNK = 512           # output-stream chunk
GSPLIT = 24            # supergroups in the first (early) all-reduce

F32 = mybir.dt.float32
F16 = mybir.dt.float16


def _mu_fallback(nc, mu8, st6):
    # mu = (me + mo) * 0.5 without an avg ALU: tensor_tensor add into mu slot,
    # then in-place scalar mult. Two Pool ops.
    nc.gpsimd.tensor_tensor(out=mu8[:, 0, :], in0=st6[:, :, 1],
                            in1=st6[:, :, 4], op=mybir.AluOpType.add)
    nc.gpsimd.tensor_scalar(out=mu8[:, 0, :], in0=mu8[:, 0, :], scalar1=0.5,
                            scalar2=None, op0=mybir.AluOpType.mult)


def build_nc(tloc=TLOC, n_cores=N_CORES, inv_alpha=1.0):
    assert tloc % (P * GRP) == 0
    nc = bacc.Bacc("TRN2", target_bir_lowering=False, debug=False,
                   num_devices=n_cores)

    x_in = nc.declare_dram_parameter("x_loc", [tloc // (P * GRP), P, GRP * C],
                                     F16, isOutput=False)  # host-repacked f16
    wq_in = nc.declare_dram_parameter("wq_g", [C, C], F32, isOutput=False)
    wk_in = nc.declare_dram_parameter("wk_g", [C, C], F32, isOutput=False)
    wvT_in = nc.declare_dram_parameter("wvT4", [C, C], F32, isOutput=False)
    wf_in = nc.declare_dram_parameter("wf", [C, C], F32, isOutput=False)
    dg_in = nc.declare_dram_parameter("diag_gamma", [C, C], F32, isOutput=False)
    idz_in = nc.declare_dram_parameter("ident_z", [P, P], F16, isOutput=False)
    id32_in = nc.declare_dram_parameter("ident_f32", [P, P], F32, isOutput=False)
    yT_out = nc.declare_dram_parameter("yT", [C, tloc], F16, isOutput=True)

    ngrp = tloc // (P * GRP)
    gsplit = min(GSPLIT, ngrp - 1) if ngrp > 1 else 1
    ntile = tloc // P
    nychunk = tloc // YCHUNK
    x_tiles = x_in.rearrange("g p (j c) -> g p j c", j=GRP)

    replica_groups = [[2 * b, 2 * b + 1] for b in range(n_cores // 2)]

    with tile.TileContext(nc) as tc:
        with (
            tc.tile_pool(name="const", bufs=1) as const,
            tc.tile_pool(name="xload", bufs=4) as xload,
            tc.tile_pool(name="stats", bufs=3) as stats,
            tc.tile_pool(name="small", bufs=2) as small,
            tc.tile_pool(name="ybuf", bufs=4) as ybuf,
            tc.tile_pool(name="psA", bufs=1, space="PSUM") as psA,
            tc.tile_pool(name="psT", bufs=2, space="PSUM") as psT,
            tc.tile_pool(name="psY", bufs=2, space="PSUM") as psY,
            tc.tile_pool(name="dram", bufs=1, space="DRAM") as dram,
        ):
            # ---- constants ----
            wq_sb = const.tile([C, C], F32)
            wk_sb = const.tile([C, C], F32)
            wvT_sb2 = const.tile([S, NH, C], F32)
            wf_sb = const.tile([C, C], F32)
            dg_sb = const.tile([C, C], F32)
            idz_sb = const.tile([P, P], F16)
            id32_sb = const.tile([P, P], F32)
            nc.sync.dma_start(out=wq_sb, in_=wq_in[:])
            nc.sync.dma_start(out=wk_sb, in_=wk_in[:])
            nc.sync.dma_start(
                out=wvT_sb2,
                in_=wvT_in[:].rearrange("(h s) c -> s h c", h=NH))
            nc.sync.dma_start(out=wf_sb, in_=wf_in[:])
            nc.sync.dma_start(out=dg_sb, in_=dg_in[:])
            nc.sync.dma_start(out=idz_sb, in_=idz_in[:])
            nc.sync.dma_start(out=id32_sb, in_=id32_in[:])
            eps_sb = const.tile([P, 1], F32)
            nc.vector.memset(eps_sb, EPS * 128.0)
            # preload the ACT exp table so phase 2 skips its ACT_TABLE_LOAD
            expwarm = const.tile([P, 1], F32)
            nc.scalar.activation(out=expwarm, in_=eps_sb,
                                 func=mybir.ActivationFunctionType.Exp,
                                 bias=0.0, scale=1.0)

            zT = const.tile([C, tloc], F16)          # transposed z stream
            Ga_ps = psA.tile([C, C], F32)            # Gram accum, sgs [0, GSPLIT)
            Gb_ps = psA.tile([C, C], F32)            # Gram accum, sgs [GSPLIT, ngrp)

            ZRING = 16
            zbig = const.tile([P, ZRING, C], F16)    # z tiles ring

            # ============ Phase 1: LN + Gram + transpose ============
            # Software-pipelined: group g's stats chain (DVE bn_stats -> Pool
            # field math -> ACT sqrt) overlaps group g-1's z-norm/Gram work,
            # so the DVE queue never stalls waiting for the chain.
            # z' = (x - mu) / std128 = z / sqrt(128); the sqrt(128) factors are
            # folded into inv_alpha (x128) and wf/diag_gamma (xsqrt(128)).
            xq = [None] * ngrp
            muq = [None] * ngrp

            def stats_chain(g):
                x8 = xload.tile([P, GRP, C], F16, tag=f"x{g % 4}")
                nc.sync.dma_start(out=x8, in_=x_tiles[g])
                st6 = stats.tile([P, GRP, 6], F32, tag=f"st{g % 3}")
                for j in range(GRP):
                    nc.vector.bn_stats(out=st6[:, j, :], in_=x8[:, j, :])
                fld = stats.tile([P, 5, GRP], F32, tag=f"fld{g % 3}")
                dif, q, dd, dd32, v128 = (fld[:, k, :] for k in range(5))
                nc.gpsimd.tensor_tensor(out=dif, in0=st6[:, :, 1],
                                        in1=st6[:, :, 4],
                                        op=mybir.AluOpType.subtract)
                nc.gpsimd.tensor_tensor(out=q, in0=st6[:, :, 2],
                                        in1=st6[:, :, 5], op=mybir.AluOpType.add)
                nc.gpsimd.tensor_tensor(out=dd, in0=dif, in1=dif,
                                        op=mybir.AluOpType.mult)
                nc.gpsimd.tensor_scalar(out=dd32, in0=dd, scalar1=32.0,
                                        scalar2=None, op0=mybir.AluOpType.mult)
                nc.gpsimd.tensor_tensor(out=v128, in0=dd32, in1=q,
                                        op=mybir.AluOpType.add)
                mu8 = stats.tile([P, 3, GRP], F32, tag=f"mu{g % 3}")
                _mu_fallback(nc, mu8, st6)
                # std128 = sqrt(v128 + 128 eps) = sqrt(128)*std  (ACT)
                nc.scalar.activation(out=mu8[:, 1, :], in_=v128,
                                     func=mybir.ActivationFunctionType.Sqrt,
                                     bias=eps_sb[:], scale=1.0)
                xq[g] = x8
                muq[g] = mu8

            def recip_step(g):
                # r128 = 1/std128 = rstd/sqrt(128)  (DVE; emitted after
                # bn_stats(g+1) so the chain latency is already hidden)
                mu8 = muq[g]
                nc.vector.reciprocal(out=mu8[:, 2, :], in_=mu8[:, 1, :])

            def norm_block(g):
                x8, mu8 = xq[g], muq[g]
                ztp = psT.tile([C, GRP * P], F16)
                for j in range(GRP):
                    i = g * GRP + j
                    z16 = zbig[:, i % ZRING, :]
                    nc.vector.tensor_scalar(
                        out=z16, in0=x8[:, j, :],
                        scalar1=mu8[:, 0, j:j + 1],
                        scalar2=mu8[:, 2, j:j + 1],
                        op0=mybir.AluOpType.subtract,
                        op1=mybir.AluOpType.mult)
                    if g < gsplit:
                        nc.tensor.matmul(Ga_ps, lhsT=z16, rhs=z16,
                                         start=(i == 0),
                                         stop=(i == gsplit * GRP - 1))
                    else:
                        nc.tensor.matmul(Gb_ps, lhsT=z16, rhs=z16,
                                         start=(i == gsplit * GRP),
                                         stop=(i == ntile - 1))
                    nc.tensor.transpose(ztp[:, j * P:(j + 1) * P], z16, idz_sb)
                nc.scalar.copy(out=zT[:, g * GRP * P:(g + 1) * GRP * P], in_=ztp)

            ar_bufs = {}

            def launch_allreduce(tag, G_ps):
                g_sb = small.tile([C, C], F32, tag=f"gsb{tag}")
                nc.vector.tensor_copy(out=g_sb, in_=G_ps)
                g_in_d = dram.tile([C, C], F32, tag=f"gin{tag}")
                g_out_d = dram.tile([C, C], F32, tag=f"gout{tag}")
                nc.gpsimd.dma_start(out=g_in_d, in_=g_sb)
                nc.gpsimd.collective_compute(
                    "AllReduce", mybir.AluOpType.add,
                    replica_groups=replica_groups,
                    ins=[g_in_d[:].opt()], outs=[g_out_d[:].opt()])
                ar_bufs[tag] = g_out_d

            stats_chain(0)
            for g in range(1, ngrp):
                stats_chain(g)
                recip_step(g - 1)
                norm_block(g - 1)
                if g == gsplit:
                    launch_allreduce("a", Ga_ps)
            recip_step(ngrp - 1)
            norm_block(ngrp - 1)
            launch_allreduce("b", Gb_ps)
            # keep the PE p-state ramped through the collective gap: a chain
            # of WAW-serialized scratch transposes (~19 us of PE busy); they
            # retire before the collective result lands, so phase 2 is not
            # delayed.
            ztp = psT.tile([C, GRP * P], F16)
            for k in range(64):
                nc.tensor.transpose(ztp[:, 0:P], zbig[:, k % ZRING, :],
                                    idz_sb)

            # ============ Phase 2: all-reduce G, softmax, W2 ============
            ga_sb = small.tile([C, C], F32)
            nc.gpsimd.dma_start(out=ga_sb, in_=ar_bufs["a"])
            gb_sb = small.tile([C, C], F32)
            nc.gpsimd.dma_start(out=gb_sb, in_=ar_bufs["b"])
            gs_sb = small.tile([C, C], F32)
            nc.vector.tensor_tensor(out=gs_sb, in0=ga_sb, in1=gb_sb,
                                    op=mybir.AluOpType.add)

            # scores_full = wk^T (G wq);  G symmetric so lhsT=G works for G@wq
            yp = psY.tile([C, 2 * YCHUNK], F32)
            s1_ps = yp[:, 0:C]
            nc.tensor.matmul(s1_ps, lhsT=gs_sb, rhs=wq_sb, start=True, stop=True)
            s1_sb = small.tile([C, C], F32)
            nc.scalar.copy(out=s1_sb, in_=s1_ps)
            sc_ps = yp[:, C:2 * C]
            nc.tensor.matmul(sc_ps, lhsT=wk_sb, rhs=s1_sb, start=True, stop=True)

            # extract 4 diagonal 32x32 blocks (scaled by 1/alpha) -> [128, 32]
            sm = small.tile([P, S], F32)
            for h in range(NH):
                nc.scalar.mul(out=sm[h * S:(h + 1) * S, :],
                              in_=sc_ps[h * S:(h + 1) * S, h * S:(h + 1) * S],
                              mul=float(inv_alpha))
            # row softmax (rows = (head, i); free = j)
            mx = small.tile([P, 1], F32)
            nc.vector.reduce_max(mx, sm, mybir.AxisListType.X)
            nmx = small.tile([P, 1], F32)
            nc.vector.tensor_scalar_mul(out=nmx, in0=mx, scalar1=-1.0)
            sh = small.tile([P, S], F32)
            nc.vector.tensor_scalar(out=sh, in0=sm, scalar1=nmx, scalar2=-87.0,
                                    op0=mybir.AluOpType.add,
                                    op1=mybir.AluOpType.max)
            ex = small.tile([P, S], F32)
            es = small.tile([P, 1], F32)
            nc.scalar.activation(out=ex, in_=sh,
                                 func=mybir.ActivationFunctionType.Exp,
                                 bias=0.0, scale=1.0, accum_out=es)
            ri = small.tile([P, 1], F32)
            nc.vector.reciprocal(out=ri, in_=es)
            at = small.tile([P, S], F32)
            nc.vector.tensor_scalar_mul(out=at, in0=ex, scalar1=ri)
            # gather per-head blocks to partitions 0..31 (cross-partition: DMA,
            # alternating issue queues)
            at4 = small.tile([S, NH, S], F32)
            for h in range(NH):
                eng = nc.sync if h % 2 == 0 else nc.gpsimd
                eng.dma_start(out=at4[:, h, :], in_=at[h * S:(h + 1) * S, :])

            # U = diag(g) Wv blockdiag(attn): per-head [128,32] matmuls
            yp = psY.tile([C, 2 * YCHUNK], F32)
            u_ps = yp[:, 0:C]
            for h in range(NH):
                nc.tensor.matmul(u_ps[:, h * S:(h + 1) * S],
                                 lhsT=wvT_sb2[:, h, :], rhs=at4[:, h, :],
                                 start=True, stop=True)
            u_sb = small.tile([C, C], F32)
            nc.scalar.copy(out=u_sb, in_=u_ps)
            ut_ps = yp[:, C:2 * C]
            nc.tensor.transpose(ut_ps, u_sb, id32_sb)
            ut_sb = small.tile([C, C], F32)
            nc.scalar.copy(out=ut_sb, in_=ut_ps)
            yp = psY.tile([C, 2 * YCHUNK], F32)
            w2_ps = yp[:, 0:C]
            nc.tensor.matmul(w2_ps, lhsT=ut_sb, rhs=wf_sb, start=True, stop=True)
            w2_sb = small.tile([C, C], F16)
            nc.vector.tensor_tensor(out=w2_sb, in0=w2_ps, in1=dg_sb,
                                    op=mybir.AluOpType.add)

            # ============ Phase 3: y^T = W2^T z^T ============
            # 1024-wide chunks: two 512-col matmuls into the two bank-halves
            # of one PSUM tile, one wide PSUM->SBUF cast (alternating DVE/ACT),
            # one wide store (alternating SP/Pool DGE queues).
            for q in range(nychunk // 2):
                yp = psY.tile([C, 2 * YCHUNK], F32)
                for h in range(2):
                    zchunk = zT[:, (2 * q + h) * YCHUNK:(2 * q + h + 1) * YCHUNK]
                    nc.tensor.matmul(yp[:, h * YCHUNK:(h + 1) * YCHUNK],
                                     lhsT=w2_sb, rhs=zchunk,
                                     start=True, stop=True)
                ys = ybuf.tile([C, 2 * YCHUNK], F16)
                if q % 2 == 0:
                    nc.scalar.copy(out=ys, in_=yp)
                else:
                    nc.vector.tensor_copy(out=ys, in_=yp)
                dma_eng = nc.sync if q % 2 == 0 else nc.gpsimd
                dma_eng.dma_start(
                    out=yT_out[:, 2 * q * YCHUNK:2 * (q + 1) * YCHUNK], in_=ys)
    nc.compile()
    return nc


def _numpy_reference(x, gamma, beta, Wq, bq, Wk, bk, Wv, bv, Wf, bf, alpha):
    """Fallback for inputs outside the zero-bias fast path."""
    Bx, Hx, Wx, Cx = x.shape
    t = Hx * Wx
    nh = NH
    s = Cx // nh
    xf = x.reshape(Bx, t, Cx).astype(np.float64)
    mu = xf.mean(-1, keepdims=True)
    var = ((xf - mu) ** 2).mean(-1, keepdims=True)
    xn = (xf - mu) / np.sqrt(var + EPS) * gamma + beta
    Q = (xn @ Wq + bq).reshape(Bx, t, nh, s)
    K = (xn @ Wk + bk).reshape(Bx, t, nh, s)
    V = (xn @ Wv + bv).reshape(Bx, t, nh, s)
    scores = np.einsum("bthi,bthj->bhij", K, Q) / float(alpha)
    scores = scores - scores.max(-1, keepdims=True)
    e = np.exp(scores)
    attn = e / e.sum(-1, keepdims=True)
    out = np.einsum("bthi,bhij->bthj", V, attn).reshape(Bx, t, Cx)
    y = out @ Wf + bf + xn
    return y.reshape(Bx, Hx, Wx, Cx).astype(np.float32)


_NC_CACHE = {}


def make_in_maps(inputs, tloc=TLOC, n_cores=N_CORES):
    x = np.asarray(inputs["x"], dtype=np.float32)
    gamma = np.asarray(inputs["gamma"], dtype=np.float32)
    Wq = np.asarray(inputs["Wq"], dtype=np.float32)
    Wk = np.asarray(inputs["Wk"], dtype=np.float32)
    Wv = np.asarray(inputs["Wv"], dtype=np.float32)
    Wf = np.ascontiguousarray(np.asarray(inputs["Wf"], dtype=np.float32))

    wq_g = np.ascontiguousarray(gamma[:, None] * Wq)
    wk_g = np.ascontiguousarray(gamma[:, None] * Wk)
    wv_g = gamma[:, None] * Wv
    wvT4 = np.ascontiguousarray(wv_g.T)
    # the device stream is z' = z/sqrt(128): fold sqrt(128) into the final
    # projection (wf) and the residual (diag gamma); scores pick up x128 via
    # inv_alpha at build time.
    rt = np.float32(np.sqrt(128.0))
    Wf = np.ascontiguousarray(Wf * rt)
    diag_g = np.ascontiguousarray((np.diag(gamma) * rt).astype(np.float32))
    ident_z = np.eye(P, dtype=np.float16)
    ident_f32 = np.eye(P, dtype=np.float32)

    ngrp = tloc // (P * GRP)
    # repack (f16) so each group load is one contiguous [P, GRP*C] 2D DMA
    xs = x.astype(np.float16).reshape(n_cores, ngrp, GRP, P, C).transpose(
        0, 1, 3, 2, 4)
    xs = np.ascontiguousarray(xs).reshape(n_cores, ngrp, P, GRP * C)
    shared = dict(wq_g=wq_g, wk_g=wk_g, wvT4=wvT4, wf=Wf, diag_gamma=diag_g,
                  ident_z=ident_z, ident_f32=ident_f32)
    return [dict(shared, x_loc=xs[i]) for i in range(n_cores)]


def kernel(**inputs) -> np.ndarray:
    zero = lambda k: not np.any(np.asarray(inputs[k]))
    if not (zero("beta") and zero("bq") and zero("bk") and zero("bv")
            and zero("bf")):
        return _numpy_reference(**{k: np.asarray(v) for k, v in inputs.items()})

    inv_alpha = 128.0 / float(np.asarray(inputs["alpha"]))
    key = ("v4", TLOC, N_CORES, inv_alpha)
    if key not in _NC_CACHE:
        _NC_CACHE[key] = build_nc(TLOC, N_CORES, inv_alpha=inv_alpha)
    nc = _NC_CACHE[key]

    in_maps = make_in_maps(inputs)
    res = run_bass_kernel_spmd(nc, in_maps, core_ids=list(range(N_CORES)))
    yT = [res.results[i]["yT"] for i in range(N_CORES)]   # each [C, TLOC] f16
    y = np.concatenate([t.T.astype(np.float32) for t in yT], axis=0)
    return np.ascontiguousarray(y.reshape(B, HH, WW, C))
